# revision 24
# baseline (speedup 1.0000x reference)
"""Trainium2 Bass kernel for nn_CopyLayer sparse_attention.

Math: the QK logit matrix of this layer is nonzero only at column 0 and the
sub-diagonal, so after causal masking softmax(qk) @ values collapses to a
closed form per row r:

    attn[r] = a0[r]*v_bos + a1[r]*values[r-1] + a2[r]*cumsum(values)[1..r]

where a0/a1/a2 are per-row softmax scalars derived from two [N]-sized dot
products (col0 = (X@qk_bos)*(X0@qk_dir), d = X@qk_previous).  The host
computes the scalars (O(B*N) work) and folds them into per-row-tile matmul
weight matrices; the device then evaluates the whole attention branch plus
the MLP branch as a chain of PE matmuls accumulating into one PSUM bank per
(row-chunk, v-half), V-major so the moving free dim is 512 rows:

    outT[v, r] = sum_kh W2T_kh[:, v]^T @ AT_kh       (MLP second layer)
              + VAz_i[:, v]^T @ combo_i              (in-tile cumsum + subdiag)
              + aux[:, v]^T @ auxw                   (cross-tile carries, BOS col)

with VAz = X*wv (row 0 zeroed), AT = relu(W1 @ X^T) kept H-major so no
transposes are needed between the MLP layers.  The strict-prefix carry
(striu) is folded into auxw on the host, so per-tile sums ts feed the aux
matmul directly.  Output is stored V-major and transposed on host.

All inputs are host-repacked so each DMA moves >=2KB contiguous packets,
and every input is fetched by ONE dma_start on the sync HWDGE queue in
exact PE-consumption order (the per-engine descriptor FIFOs then give the
early tensors full bandwidth); outputs go on the scalar HWDGE queue.

Sharding: data-parallel over batch B=8, one batch per NeuronCore (8 cores).
"""

import numpy as np

B, N, V, H = 8, 2048, 256, 1024
P, T, RC = 128, 16, 4
EPS = 1e-5

# set by test harness: 0 = no trace, 1 = trace core 0
KERNEL_TRACE = False
last_exec_time_ns = None
last_results = None

_module_cache = {}

USE_F32R = False  # exact-fp32 matmul variant (4x slower); fp16 path is the default


def _build_module(use_f32r):
    import concourse.bacc as bacc
    import concourse.tile as tile
    from concourse import mybir
    from contextlib import ExitStack

    dt = mybir.dt
    f32 = dt.float32
    bf16 = dt.float16
    mmdt = dt.float32r

    nc = bacc.Bacc("TRN2", enable_partition_id=False)
    c0_d = nc.dram_tensor("c0", [P, 3 * V], bf16, kind="ExternalInput")
    x_d = nc.dram_tensor("x", [P, T * V], bf16, kind="ExternalInput")
    xt_d = nc.dram_tensor("xt", [P, RC, 2, 512], bf16, kind="ExternalInput")
    w1t_d = nc.dram_tensor("w1t", [P, 4, 2, 256], bf16, kind="ExternalInput")
    wc_d = nc.dram_tensor("wc", [P, 8 * V + T * P], bf16, kind="ExternalInput")
    auxw_d = nc.dram_tensor("auxw", [64, T * P], bf16, kind="ExternalInput")
    out_d = nc.dram_tensor("out", [P, RC, 2, 512], bf16, kind="ExternalOutput")

    def mm(ap):
        return ap.bitcast(mmdt) if use_f32r else ap

    with tile.TileContext(nc) as tc, ExitStack() as ctx:
        consts = ctx.enter_context(tc.tile_pool(name="consts", bufs=1))
        big = ctx.enter_context(tc.tile_pool(name="big", bufs=1))
        atp = ctx.enter_context(tc.tile_pool(name="atp", bufs=3))
        outp = ctx.enter_context(tc.tile_pool(name="outp", bufs=2))
        pt = ctx.enter_context(tc.tile_pool(name="pt", bufs=3, space="PSUM"))
        pa = ctx.enter_context(tc.tile_pool(name="pa", bufs=3, space="PSUM"))
        ps = ctx.enter_context(tc.tile_pool(name="ps", bufs=1, space="PSUM"))

        # ---- HAM warmup: junk matmuls while the first DMAs land ----
        warm_sb = consts.tile([P, 512], bf16)
        nc.gpsimd.memset(warm_sb, 0.0)
        for _w in range(8):
            wp = pa.tile([P, 512], f32, tag="a_ps")
            nc.tensor.matmul(wp, mm(warm_sb[:, 0:128]), mm(warm_sb),
                             start=True, stop=True)
        for _w in range(8):
            wp = pa.tile([P, 512], f32, tag="a_ps")
            nc.tensor.matmul(wp[:, 0:128], mm(warm_sb[:, 0:128]),
                             mm(warm_sb[:, 0:128]), start=True, stop=True)

        # ---- inputs: two parallel ordered streams.  Sync queue: w1t first,
        #      then the attention feed; scalar queue: xt0/xt1 so the MLP1
        #      moving operand loads in parallel with its weights.  Both in
        #      PE-consumption order. ----
        w1t_sb = consts.tile([P, 4, 2, 256], bf16)
        nc.sync.dma_start(out=w1t_sb[:, 0, :, :], in_=w1t_d[:, 0, :, :])
        xt0_sb = big.tile([P, 2, 512], bf16, tag="xt0")
        xt_sbs = [xt0_sb]
        nc.scalar.dma_start(out=xt0_sb[:, 0, :], in_=xt_d[:, 0, 0, :])
        nc.scalar.dma_start(out=xt0_sb[:, 1, :], in_=xt_d[:, 0, 1, :])
        nc.sync.dma_start(out=w1t_sb[:, 1:4, :, :], in_=w1t_d[:, 1:4, :, :])
        xt1_sb = big.tile([P, 2, 512], bf16, tag="xt1")
        nc.scalar.dma_start(out=xt1_sb, in_=xt_d[:, 1, :, :])
        xt_sbs.append(xt1_sb)
        c0_sb = consts.tile([P, 3 * V], bf16)
        nc.sync.dma_start(out=c0_sb, in_=c0_d[:])
        ohc_v = c0_sb[:, 0:V]                      # [128, 16*16] one-hot colsum
        wvb_v = c0_sb[:, V:2 * V]                  # [128, 256] wv broadcast
        aux_v = c0_sb[:, 2 * V:3 * V]              # [64.., 256] aux rows
        x_sb = big.tile([P, T, V], bf16)
        nc.sync.dma_start(out=x_sb[:, 0:8, :],
                          in_=x_d[:, 0:8 * V].rearrange("p (t v) -> p t v", t=8))
        nc.sync.dma_start(out=x_sb[:, 8:16, :],
                          in_=x_d[:, 8 * V:].rearrange("p (t v) -> p t v", t=8))
        wc_sb = consts.tile([P, 8 * V + T * P], bf16)
        nc.sync.dma_start(out=wc_sb, in_=wc_d[:])
        w2t_sb = wc_sb[:, 0:8 * V].rearrange("p (k v) -> p k v", k=8)
        combo_sb = wc_sb[:, 8 * V:].rearrange("p (t r) -> p t r", t=T)
        auxw_sb = consts.tile([64, T, P], bf16)
        nc.sync.dma_start(out=auxw_sb, in_=auxw_d[:].rearrange("p (t r) -> p t r", t=T))
        for rc in range(2, RC):
            xt_rc = big.tile([P, 2, 512], bf16, tag=f"xt{rc}")
            nc.sync.dma_start(out=xt_rc, in_=xt_d[:, rc, :, :])
            xt_sbs.append(xt_rc)

        # ---- MLP layer 1: AT = relu(W1 @ X^T), H-major [128, kh, 512] ----
        at_sbs = [None] * RC

        def mm1(rc):
            at_sb = atp.tile([P, 8, 512], bf16)
            for kh in range(8):
                a_ps = pa.tile([P, 512], f32)
                for kv in range(2):
                    nc.tensor.matmul(
                        a_ps,
                        mm(w1t_sb[:, kh // 2, kv, (kh % 2) * P:(kh % 2 + 1) * P]),
                        mm(xt_sbs[rc][:, kv, :]),
                        start=(kv == 0), stop=(kv == 1))
                if kh % 2 == 0:
                    nc.scalar.activation(out=at_sb[:, kh, :], in_=a_ps,
                                         func=mybir.ActivationFunctionType.Relu)
                else:
                    nc.vector.tensor_scalar_max(at_sb[:, kh, :], a_ps, 0.0)
            at_sbs[rc] = at_sb

        mm1(0)
        mm1(1)

        # ---- VAz = x * wv, row 0 zeroed (emitted after mm1 relus so the
        #      DVE queue doesn't head-of-line block on the x DMA; split
        #      across DVE and GpSimd so neither serializes the ts chain) ----
        vaz = big.tile([P, T, V], bf16)
        for i in range(T):
            if i % 3 == 2:
                nc.gpsimd.tensor_mul(vaz[:, i, :], x_sb[:, i, :], wvb_v)
            else:
                nc.vector.tensor_mul(vaz[:, i, :], x_sb[:, i, :], wvb_v)
        nc.vector.memset(vaz[0:1, 0, :], 0.0)

        # ---- per-tile sums ts -> aux rows 0..15 (striu folded into auxw) ----
        ts_ps = ps.tile([16, V], f32)
        for i in range(T):
            nc.tensor.matmul(ts_ps, mm(ohc_v[:, i * 16:(i + 1) * 16]),
                             mm(vaz[:, i, :]), start=(i == 0), stop=(i == T - 1))
        nc.vector.tensor_copy(aux_v[0:16, :], ts_ps)

        # ---- fused V-major attention + MLP-2 accumulation per (rc, vc) ----
        def chain(rc, vc):
            at_sb = at_sbs[rc]
            o_ps = pt.tile([P, 512], f32)
            for kh in range(8):
                nc.tensor.matmul(o_ps,
                                 mm(w2t_sb[:, kh, vc * P:(vc + 1) * P]),
                                 mm(at_sb[:, kh, :]),
                                 start=(kh == 0), stop=False)
            for j in range(4):
                i = rc * 4 + j
                nc.tensor.matmul(o_ps[:, j * P:(j + 1) * P],
                                 mm(vaz[:, i, vc * P:(vc + 1) * P]),
                                 mm(combo_sb[:, i, :]),
                                 start=False, stop=False)
            nc.tensor.matmul(o_ps,
                             mm(aux_v[0:64, vc * P:(vc + 1) * P]),
                             mm(auxw_sb[:, rc * 4:(rc + 1) * 4, :]),
                             start=False, stop=True)
            return o_ps

        for rc in range(RC):
            if rc >= 2:
                mm1(rc)
            o_rc = outp.tile([P, 2, 512], bf16)
            for vc in range(2):
                o_ps = chain(rc, vc)
                nc.vector.tensor_copy(o_rc[:, vc, :], o_ps)
            nc.scalar.dma_start(out=out_d[:, rc, :, :], in_=o_rc)
    nc.compile()
    return nc


def _get_module():
    key = ("mod", USE_F32R)
    if key not in _module_cache:
        _module_cache[key] = _build_module(USE_F32R)
    return _module_cache[key]


def _ln(x, g, b):
    m = x.mean(-1, keepdims=True)
    v = ((x - m) ** 2).mean(-1, keepdims=True)
    return (x - m) / np.sqrt(v + EPS) * g + b


def _is_tril_masks(mask_one, mask_zero):
    if mask_one.shape != (N, N) or mask_zero.shape != (N, N):
        return False
    tril = np.tril(np.ones((N, N), np.float32))
    return (np.array_equal(mask_one, tril)
            and np.array_equal(mask_zero, np.float32(-1e9) * (1.0 - tril)))


def _dense_fallback(h, mask_one, mask_zero, ln_attn_g, ln_attn_b, ln_mlp_g,
                    ln_mlp_b, wv, wv_bos, wo_w, qk_bos, qk_previous,
                    qk_direction, w1, w2):
    """Faithful numpy port of the reference for arbitrary masks."""
    b, n, v = h.shape
    attn_input = h.copy()
    attn_input[:, 0, :] = _ln(h[:, 0, :], ln_attn_g, ln_attn_b)
    values = attn_input[:, 1:, :] * wv
    v_bos = wo_w @ wv_bos
    values = np.concatenate(
        [np.broadcast_to(v_bos, (b, 1, v)), values], axis=1)
    col0 = (attn_input @ qk_bos) * (attn_input[:, 0, :] @ qk_direction)[:, None]
    d = attn_input @ qk_previous
    out = np.empty_like(h)
    idx = np.arange(1, n)
    for bi in range(b):
        qk = np.zeros((n, n), np.float32)
        qk[:, 0] += col0[bi]
        qk[idx, idx - 1] += d[bi, 1:]
        qk = qk * mask_one + mask_zero
        qk -= qk.max(axis=-1, keepdims=True)
        e = np.exp(qk)
        p = e / e.sum(axis=-1, keepdims=True)
        out[bi] = p @ values[bi]
    mlp_input = h.copy()
    mlp_input[:, 0, :] = _ln(h[:, 0, :], ln_mlp_g, ln_mlp_b)
    out += np.maximum(mlp_input @ w1.T, 0.0) @ w2.T
    return out


def kernel(h, mask_one, mask_zero, ln_attn_g, ln_attn_b, ln_mlp_g, ln_mlp_b,
           wv, wv_bos, wo_w, qk_bos, qk_previous, qk_direction, w1, w2):
    global last_exec_time_ns, last_results
    h = np.ascontiguousarray(np.asarray(h, np.float32))
    mask_one = np.asarray(mask_one, np.float32)
    mask_zero = np.asarray(mask_zero, np.float32)
    ln_attn_g = np.asarray(ln_attn_g, np.float32)
    ln_attn_b = np.asarray(ln_attn_b, np.float32)
    ln_mlp_g = np.asarray(ln_mlp_g, np.float32)
    ln_mlp_b = np.asarray(ln_mlp_b, np.float32)
    wv = np.asarray(wv, np.float32)
    wv_bos = np.asarray(wv_bos, np.float32)
    wo_w = np.asarray(wo_w, np.float32)
    qk_bos = np.asarray(qk_bos, np.float32)
    qk_previous = np.asarray(qk_previous, np.float32)
    qk_direction = np.asarray(qk_direction, np.float32)
    w1 = np.asarray(w1, np.float32)
    w2 = np.asarray(w2, np.float32)

    if h.shape != (B, N, V) or not _is_tril_masks(mask_one, mask_zero):
        return _dense_fallback(h, mask_one, mask_zero, ln_attn_g, ln_attn_b,
                               ln_mlp_g, ln_mlp_b, wv, wv_bos, wo_w, qk_bos,
                               qk_previous, qk_direction, w1, w2)

    from concourse.bass_utils import run_bass_kernel_spmd

    in_maps, v_bos, mlp_row0 = _prepare(
        h, ln_attn_g, ln_attn_b, ln_mlp_g, ln_mlp_b, wv, wv_bos, wo_w,
        qk_bos, qk_previous, qk_direction, w1, w2)

    nc = _get_module()
    res = run_bass_kernel_spmd(nc, in_maps, core_ids=list(range(B)),
                               trace=bool(KERNEL_TRACE))
    last_exec_time_ns = res.exec_time_ns
    last_results = res

    # ---- host epilogue: gather (V-major -> row-major) + row-0 fix ----
    out = np.empty((B, N, V), np.float32)
    for b in range(B):
        o = res.results[b]["out"].astype(np.float32)      # [128, RC, 2, 512]
        out[b] = o.transpose(1, 3, 2, 0).reshape(N, V)
        out[b, 0] = v_bos + mlp_row0[b]
    return out


def _prepare(h, ln_attn_g, ln_attn_b, ln_mlp_g, ln_mlp_b, wv, wv_bos, wo_w,
             qk_bos, qk_previous, qk_direction, w1, w2):
    # ---- shared host precompute ----
    bf16 = np.float16
    v_bos = (wo_w @ wv_bos).astype(np.float32)
    w1t = np.ascontiguousarray(w1.T)
    w2t = np.ascontiguousarray(w2.T)
    # repack weights so DMA rows are >=2KB contiguous
    # w1t split into four quarter-H chunks: [p, q, kv, hq]
    w1t_b = np.ascontiguousarray(
        w1t.reshape(2, P, 4, 256).transpose(1, 2, 0, 3)).astype(bf16)
    w2t_b = np.ascontiguousarray(
        w2t.reshape(8, P, V).transpose(1, 0, 2).reshape(P, 8 * V)).astype(bf16)
    wvb = np.ascontiguousarray(np.broadcast_to(wv, (P, V))).astype(np.float32)
    ohc = np.zeros((P, T, T), np.float32)
    for i in range(T):
        ohc[:, i, i] = 1.0
    ohc = ohc.reshape(P, T * T)

    attn0 = _ln(h[:, 0, :].astype(np.float64), ln_attn_g, ln_attn_b).astype(np.float32)
    mlp0 = _ln(h[:, 0, :].astype(np.float64), ln_mlp_g, ln_mlp_b).astype(np.float32)

    cc = np.arange(P)
    le = (cc[:, None] <= cc[None, :]).astype(np.float32)   # [c, r]
    rr = np.arange(N)

    in_maps = []
    for b in range(B):
        X = h[b].copy()
        X[0] = attn0[b]
        s_b = float(attn0[b].astype(np.float64) @ qk_direction)
        qk2 = np.stack([qk_bos * np.float32(s_b), qk_previous], axis=1)  # [V, 2]
        cd = X.astype(np.float64) @ qk2.astype(np.float64)               # [N, 2]
        col0, d = cd[:, 0], cd[:, 1]
        ce = col0.copy()
        ce[1] = col0[1] + d[1]
        de = np.where(rr >= 2, d, -1e30)
        cnt = np.where(rr == 0, 0.0, np.where(rr == 1, 1.0, rr - 1.0))
        m = np.maximum(np.maximum(ce, de), 0.0)
        e0 = np.exp(ce - m)
        ed = np.exp(de - m)
        ez = np.exp(-m)
        sub = (rr >= 2).astype(np.float64)
        Z = e0 + ed + cnt * ez
        a0 = (e0 / Z).astype(np.float32)
        a1 = ((ed - sub * ez) / Z).astype(np.float32)
        a2 = (ez / Z).astype(np.float32)

        a0t = a0.reshape(T, P)
        a1t = a1.reshape(T, P)
        a2t = a2.reshape(T, P)
        # combo[c, i, r] = a2[i,r] * (c <= r) + a1[i,r] * (c == r-1)
        combo = a2t[:, None, :] * le[None, :, :]             # [T, c, r]
        combo[:, cc[:-1], cc[1:]] += a1t[:, 1:]
        combo = np.ascontiguousarray(
            combo.transpose(1, 0, 2).reshape(P, T * P)).astype(bf16)

        # auxw[c, i, r]: carries (striu folded: ts[j] feeds all tiles i > j),
        # previous-tile last row for each tile's first row, BOS column term
        auxw = np.zeros((64, T, P), np.float32)
        for i in range(T):
            auxw[:i, i, :] = a2t[i]
            if i >= 1:
                auxw[16 + i - 1, i, 0] = a1t[i, 0]
            auxw[32, i, :] = a0t[i]
        auxw = auxw.reshape(64, T * P).astype(bf16)

        aux0 = np.zeros((P, V), np.float32)
        lastrows = h[b, 127::128, :][:15] * wv               # VA[128j+127]
        aux0[16:16 + 15] = lastrows
        aux0[32] = v_bos

        c0 = np.concatenate([ohc, wvb, aux0], axis=1)        # [P, 3V]

        # x: [p, t, v] so DMA rows are 8KB; xt: [p, rc, kv, r] so each
        # row-chunk slice is 2KB-contiguous
        x2 = np.ascontiguousarray(
            X.reshape(T, P, V).transpose(1, 0, 2).reshape(P, T * V))
        xt4 = np.ascontiguousarray(
            X.T.reshape(2, P, RC, 512).transpose(1, 2, 0, 3))
        wc = np.concatenate([w2t_b, combo], axis=1)          # [P, 8V + T*P]

        in_maps.append({
            "c0": c0.astype(bf16),
            "x": x2.astype(bf16),
            "xt": xt4.astype(bf16),
            "w1t": w1t_b,
            "wc": wc,
            "auxw": auxw,
        })

    mlp_row0 = np.maximum(mlp0 @ w1t, 0.0) @ w2t             # [B, V]
    return in_maps, v_bos, mlp_row0


# revision 26
# speedup vs baseline: 1.0248x; 1.0248x over previous
"""Trainium2 Bass kernel for nn_CopyLayer sparse_attention.

Math: the QK logit matrix of this layer is nonzero only at column 0 and the
sub-diagonal, so after causal masking softmax(qk) @ values collapses to a
closed form per row r:

    attn[r] = a0[r]*v_bos + a1[r]*values[r-1] + a2[r]*cumsum(values)[1..r]

where a0/a1/a2 are per-row softmax scalars derived from two [N]-sized dot
products (col0 = (X@qk_bos)*(X0@qk_dir), d = X@qk_previous).  The host
computes the scalars (O(B*N) work) and folds them into per-row-tile matmul
weight matrices; the device then evaluates the whole attention branch plus
the MLP branch as a chain of PE matmuls accumulating into one PSUM bank per
(row-chunk, v-half), V-major so the moving free dim is 512 rows:

    outT[v, r] = sum_kh W2T_kh[:, v]^T @ AT_kh       (MLP second layer)
              + VAz_i[:, v]^T @ combo_i              (in-tile cumsum + subdiag)
              + aux[:, v]^T @ auxw                   (cross-tile carries, BOS col)

with VAz = X*wv (row 0 zeroed), AT = relu(W1 @ X^T) kept H-major so no
transposes are needed between the MLP layers.  The strict-prefix carry
(striu) is folded into auxw on the host, so per-tile sums ts feed the aux
matmul directly.  Output is stored V-major and transposed on host.

All inputs are host-repacked so each DMA moves >=2KB contiguous packets,
and every input is fetched by ONE dma_start on the sync HWDGE queue in
exact PE-consumption order (the per-engine descriptor FIFOs then give the
early tensors full bandwidth); outputs go on the scalar HWDGE queue.

Sharding: data-parallel over batch B=8, one batch per NeuronCore (8 cores).
"""

import numpy as np

B, N, V, H = 8, 2048, 256, 1024
P, T, RC = 128, 16, 4
EPS = 1e-5

# set by test harness: 0 = no trace, 1 = trace core 0
KERNEL_TRACE = False
last_exec_time_ns = None
last_results = None

_module_cache = {}

USE_F32R = False  # exact-fp32 matmul variant (4x slower); fp16 path is the default


def _build_module(use_f32r):
    import concourse.bacc as bacc
    import concourse.tile as tile
    from concourse import mybir
    from contextlib import ExitStack

    dt = mybir.dt
    f32 = dt.float32
    bf16 = dt.float16
    mmdt = dt.float32r

    nc = bacc.Bacc("TRN2", enable_partition_id=False)
    c0_d = nc.dram_tensor("c0", [P, 3 * V], bf16, kind="ExternalInput")
    x_d = nc.dram_tensor("x", [P, T * V], bf16, kind="ExternalInput")
    xt_d = nc.dram_tensor("xt", [P, RC, 2, 512], bf16, kind="ExternalInput")
    w1t_d = nc.dram_tensor("w1t", [P, 4, 2, 256], bf16, kind="ExternalInput")
    wc_d = nc.dram_tensor("wc", [P, 8 * V + T * P], bf16, kind="ExternalInput")
    auxw_d = nc.dram_tensor("auxw", [64, T * P], bf16, kind="ExternalInput")
    out_d = nc.dram_tensor("out", [P, RC, 2, 512], bf16, kind="ExternalOutput")

    def mm(ap):
        return ap.bitcast(mmdt) if use_f32r else ap

    with tile.TileContext(nc) as tc, ExitStack() as ctx:
        consts = ctx.enter_context(tc.tile_pool(name="consts", bufs=1))
        big = ctx.enter_context(tc.tile_pool(name="big", bufs=1))
        atp = ctx.enter_context(tc.tile_pool(name="atp", bufs=3))
        outp = ctx.enter_context(tc.tile_pool(name="outp", bufs=2))
        pt = ctx.enter_context(tc.tile_pool(name="pt", bufs=3, space="PSUM"))
        pa = ctx.enter_context(tc.tile_pool(name="pa", bufs=3, space="PSUM"))
        ps = ctx.enter_context(tc.tile_pool(name="ps", bufs=1, space="PSUM"))

        # ---- HAM warmup: junk matmuls while the first DMAs land ----
        warm_sb = consts.tile([P, 512], bf16)
        nc.gpsimd.memset(warm_sb, 0.0)
        for _w in range(7):
            wp = pa.tile([P, 512], f32, tag="a_ps")
            nc.tensor.matmul(wp, mm(warm_sb[:, 0:128]), mm(warm_sb),
                             start=True, stop=True)
        for _w in range(4):
            wp = pa.tile([P, 512], f32, tag="a_ps")
            nc.tensor.matmul(wp[:, 0:128], mm(warm_sb[:, 0:128]),
                             mm(warm_sb[:, 0:128]), start=True, stop=True)

        # ---- inputs: two parallel ordered streams.  Sync queue: w1t first,
        #      then the attention feed; scalar queue: xt0/xt1 so the MLP1
        #      moving operand loads in parallel with its weights.  Both in
        #      PE-consumption order. ----
        w1t_sb = consts.tile([P, 4, 2, 256], bf16)
        nc.sync.dma_start(out=w1t_sb[:, 0, :, :], in_=w1t_d[:, 0, :, :])
        xt0_sb = big.tile([P, 2, 512], bf16, tag="xt0")
        xt_sbs = [xt0_sb]
        nc.sync.dma_start(out=xt0_sb[:, 0, :], in_=xt_d[:, 0, 0, :])
        nc.sync.dma_start(out=xt0_sb[:, 1, :], in_=xt_d[:, 0, 1, :])
        nc.sync.dma_start(out=w1t_sb[:, 1, :, :], in_=w1t_d[:, 1, :, :])
        nc.sync.dma_start(out=w1t_sb[:, 2, :, :], in_=w1t_d[:, 2, :, :])
        nc.sync.dma_start(out=w1t_sb[:, 3, :, :], in_=w1t_d[:, 3, :, :])
        xt1_sb = big.tile([P, 2, 512], bf16, tag="xt1")
        nc.sync.dma_start(out=xt1_sb, in_=xt_d[:, 1, :, :])
        xt_sbs.append(xt1_sb)
        c0_sb = consts.tile([P, 3 * V], bf16)
        nc.sync.dma_start(out=c0_sb, in_=c0_d[:])
        ohc_v = c0_sb[:, 0:V]                      # [128, 16*16] one-hot colsum
        wvb_v = c0_sb[:, V:2 * V]                  # [128, 256] wv broadcast
        aux_v = c0_sb[:, 2 * V:3 * V]              # [64.., 256] aux rows
        x_sb = big.tile([P, T, V], bf16)
        nc.sync.dma_start(out=x_sb[:, 0:8, :],
                          in_=x_d[:, 0:8 * V].rearrange("p (t v) -> p t v", t=8))
        nc.sync.dma_start(out=x_sb[:, 8:16, :],
                          in_=x_d[:, 8 * V:].rearrange("p (t v) -> p t v", t=8))
        wc_sb = consts.tile([P, 8 * V + T * P], bf16)
        nc.sync.dma_start(out=wc_sb, in_=wc_d[:])
        w2t_sb = wc_sb[:, 0:8 * V].rearrange("p (k v) -> p k v", k=8)
        combo_sb = wc_sb[:, 8 * V:].rearrange("p (t r) -> p t r", t=T)
        auxw_sb = consts.tile([64, T, P], bf16)
        nc.sync.dma_start(out=auxw_sb, in_=auxw_d[:].rearrange("p (t r) -> p t r", t=T))
        for rc in range(2, RC):
            xt_rc = big.tile([P, 2, 512], bf16, tag=f"xt{rc}")
            nc.sync.dma_start(out=xt_rc, in_=xt_d[:, rc, :, :])
            xt_sbs.append(xt_rc)

        # ---- MLP layer 1: AT = relu(W1 @ X^T), H-major [128, kh, 512] ----
        at_sbs = [None] * RC

        def mm1(rc):
            at_sb = atp.tile([P, 8, 512], bf16)
            for kh in range(8):
                a_ps = pa.tile([P, 512], f32)
                for kv in range(2):
                    nc.tensor.matmul(
                        a_ps,
                        mm(w1t_sb[:, kh // 2, kv, (kh % 2) * P:(kh % 2 + 1) * P]),
                        mm(xt_sbs[rc][:, kv, :]),
                        start=(kv == 0), stop=(kv == 1))
                if kh % 2 == 0:
                    nc.scalar.activation(out=at_sb[:, kh, :], in_=a_ps,
                                         func=mybir.ActivationFunctionType.Relu)
                else:
                    nc.vector.tensor_scalar_max(at_sb[:, kh, :], a_ps, 0.0)
            at_sbs[rc] = at_sb

        mm1(0)
        mm1(1)

        # ---- VAz = x * wv, row 0 zeroed (emitted after mm1 relus so the
        #      DVE queue doesn't head-of-line block on the x DMA; split
        #      across DVE and GpSimd so neither serializes the ts chain) ----
        vaz = big.tile([P, T, V], bf16)
        for i in range(T):
            if i % 3 == 2:
                nc.gpsimd.tensor_mul(vaz[:, i, :], x_sb[:, i, :], wvb_v)
            else:
                nc.vector.tensor_mul(vaz[:, i, :], x_sb[:, i, :], wvb_v)
        nc.vector.memset(vaz[0:1, 0, :], 0.0)

        # ---- per-tile sums ts -> aux rows 0..15 (striu folded into auxw) ----
        ts_ps = ps.tile([16, V], f32)
        for i in range(T):
            nc.tensor.matmul(ts_ps, mm(ohc_v[:, i * 16:(i + 1) * 16]),
                             mm(vaz[:, i, :]), start=(i == 0), stop=(i == T - 1))
        nc.vector.tensor_copy(aux_v[0:16, :], ts_ps)

        # ---- fused V-major attention + MLP-2 accumulation per (rc, vc) ----
        def chain(rc, vc):
            at_sb = at_sbs[rc]
            o_ps = pt.tile([P, 512], f32)
            for kh in range(8):
                nc.tensor.matmul(o_ps,
                                 mm(w2t_sb[:, kh, vc * P:(vc + 1) * P]),
                                 mm(at_sb[:, kh, :]),
                                 start=(kh == 0), stop=False)
            for j in range(4):
                i = rc * 4 + j
                nc.tensor.matmul(o_ps[:, j * P:(j + 1) * P],
                                 mm(vaz[:, i, vc * P:(vc + 1) * P]),
                                 mm(combo_sb[:, i, :]),
                                 start=False, stop=False)
            nc.tensor.matmul(o_ps,
                             mm(aux_v[0:64, vc * P:(vc + 1) * P]),
                             mm(auxw_sb[:, rc * 4:(rc + 1) * 4, :]),
                             start=False, stop=True)
            return o_ps

        for rc in range(RC):
            if rc >= 2:
                mm1(rc)
            o_rc = outp.tile([P, 2, 512], bf16)
            for vc in range(2):
                o_ps = chain(rc, vc)
                nc.vector.tensor_copy(o_rc[:, vc, :], o_ps)
            nc.scalar.dma_start(out=out_d[:, rc, :, :], in_=o_rc)
    nc.compile()
    return nc


def _get_module():
    key = ("mod", USE_F32R)
    if key not in _module_cache:
        _module_cache[key] = _build_module(USE_F32R)
    return _module_cache[key]


def _ln(x, g, b):
    m = x.mean(-1, keepdims=True)
    v = ((x - m) ** 2).mean(-1, keepdims=True)
    return (x - m) / np.sqrt(v + EPS) * g + b


def _is_tril_masks(mask_one, mask_zero):
    if mask_one.shape != (N, N) or mask_zero.shape != (N, N):
        return False
    tril = np.tril(np.ones((N, N), np.float32))
    return (np.array_equal(mask_one, tril)
            and np.array_equal(mask_zero, np.float32(-1e9) * (1.0 - tril)))


def _dense_fallback(h, mask_one, mask_zero, ln_attn_g, ln_attn_b, ln_mlp_g,
                    ln_mlp_b, wv, wv_bos, wo_w, qk_bos, qk_previous,
                    qk_direction, w1, w2):
    """Faithful numpy port of the reference for arbitrary masks."""
    b, n, v = h.shape
    attn_input = h.copy()
    attn_input[:, 0, :] = _ln(h[:, 0, :], ln_attn_g, ln_attn_b)
    values = attn_input[:, 1:, :] * wv
    v_bos = wo_w @ wv_bos
    values = np.concatenate(
        [np.broadcast_to(v_bos, (b, 1, v)), values], axis=1)
    col0 = (attn_input @ qk_bos) * (attn_input[:, 0, :] @ qk_direction)[:, None]
    d = attn_input @ qk_previous
    out = np.empty_like(h)
    idx = np.arange(1, n)
    for bi in range(b):
        qk = np.zeros((n, n), np.float32)
        qk[:, 0] += col0[bi]
        qk[idx, idx - 1] += d[bi, 1:]
        qk = qk * mask_one + mask_zero
        qk -= qk.max(axis=-1, keepdims=True)
        e = np.exp(qk)
        p = e / e.sum(axis=-1, keepdims=True)
        out[bi] = p @ values[bi]
    mlp_input = h.copy()
    mlp_input[:, 0, :] = _ln(h[:, 0, :], ln_mlp_g, ln_mlp_b)
    out += np.maximum(mlp_input @ w1.T, 0.0) @ w2.T
    return out


def kernel(h, mask_one, mask_zero, ln_attn_g, ln_attn_b, ln_mlp_g, ln_mlp_b,
           wv, wv_bos, wo_w, qk_bos, qk_previous, qk_direction, w1, w2):
    global last_exec_time_ns, last_results
    h = np.ascontiguousarray(np.asarray(h, np.float32))
    mask_one = np.asarray(mask_one, np.float32)
    mask_zero = np.asarray(mask_zero, np.float32)
    ln_attn_g = np.asarray(ln_attn_g, np.float32)
    ln_attn_b = np.asarray(ln_attn_b, np.float32)
    ln_mlp_g = np.asarray(ln_mlp_g, np.float32)
    ln_mlp_b = np.asarray(ln_mlp_b, np.float32)
    wv = np.asarray(wv, np.float32)
    wv_bos = np.asarray(wv_bos, np.float32)
    wo_w = np.asarray(wo_w, np.float32)
    qk_bos = np.asarray(qk_bos, np.float32)
    qk_previous = np.asarray(qk_previous, np.float32)
    qk_direction = np.asarray(qk_direction, np.float32)
    w1 = np.asarray(w1, np.float32)
    w2 = np.asarray(w2, np.float32)

    if h.shape != (B, N, V) or not _is_tril_masks(mask_one, mask_zero):
        return _dense_fallback(h, mask_one, mask_zero, ln_attn_g, ln_attn_b,
                               ln_mlp_g, ln_mlp_b, wv, wv_bos, wo_w, qk_bos,
                               qk_previous, qk_direction, w1, w2)

    from concourse.bass_utils import run_bass_kernel_spmd

    in_maps, v_bos, mlp_row0 = _prepare(
        h, ln_attn_g, ln_attn_b, ln_mlp_g, ln_mlp_b, wv, wv_bos, wo_w,
        qk_bos, qk_previous, qk_direction, w1, w2)

    nc = _get_module()
    res = run_bass_kernel_spmd(nc, in_maps, core_ids=list(range(B)),
                               trace=bool(KERNEL_TRACE))
    last_exec_time_ns = res.exec_time_ns
    last_results = res

    # ---- host epilogue: gather (V-major -> row-major) + row-0 fix ----
    out = np.empty((B, N, V), np.float32)
    for b in range(B):
        o = res.results[b]["out"].astype(np.float32)      # [128, RC, 2, 512]
        out[b] = o.transpose(1, 3, 2, 0).reshape(N, V)
        out[b, 0] = v_bos + mlp_row0[b]
    return out


def _prepare(h, ln_attn_g, ln_attn_b, ln_mlp_g, ln_mlp_b, wv, wv_bos, wo_w,
             qk_bos, qk_previous, qk_direction, w1, w2):
    # ---- shared host precompute ----
    bf16 = np.float16
    v_bos = (wo_w @ wv_bos).astype(np.float32)
    w1t = np.ascontiguousarray(w1.T)
    w2t = np.ascontiguousarray(w2.T)
    # repack weights so DMA rows are >=2KB contiguous
    # w1t split into four quarter-H chunks: [p, q, kv, hq]
    w1t_b = np.ascontiguousarray(
        w1t.reshape(2, P, 4, 256).transpose(1, 2, 0, 3)).astype(bf16)
    w2t_b = np.ascontiguousarray(
        w2t.reshape(8, P, V).transpose(1, 0, 2).reshape(P, 8 * V)).astype(bf16)
    wvb = np.ascontiguousarray(np.broadcast_to(wv, (P, V))).astype(np.float32)
    ohc = np.zeros((P, T, T), np.float32)
    for i in range(T):
        ohc[:, i, i] = 1.0
    ohc = ohc.reshape(P, T * T)

    attn0 = _ln(h[:, 0, :].astype(np.float64), ln_attn_g, ln_attn_b).astype(np.float32)
    mlp0 = _ln(h[:, 0, :].astype(np.float64), ln_mlp_g, ln_mlp_b).astype(np.float32)

    cc = np.arange(P)
    le = (cc[:, None] <= cc[None, :]).astype(np.float32)   # [c, r]
    rr = np.arange(N)

    in_maps = []
    for b in range(B):
        X = h[b].copy()
        X[0] = attn0[b]
        s_b = float(attn0[b].astype(np.float64) @ qk_direction)
        qk2 = np.stack([qk_bos * np.float32(s_b), qk_previous], axis=1)  # [V, 2]
        cd = X.astype(np.float64) @ qk2.astype(np.float64)               # [N, 2]
        col0, d = cd[:, 0], cd[:, 1]
        ce = col0.copy()
        ce[1] = col0[1] + d[1]
        de = np.where(rr >= 2, d, -1e30)
        cnt = np.where(rr == 0, 0.0, np.where(rr == 1, 1.0, rr - 1.0))
        m = np.maximum(np.maximum(ce, de), 0.0)
        e0 = np.exp(ce - m)
        ed = np.exp(de - m)
        ez = np.exp(-m)
        sub = (rr >= 2).astype(np.float64)
        Z = e0 + ed + cnt * ez
        a0 = (e0 / Z).astype(np.float32)
        a1 = ((ed - sub * ez) / Z).astype(np.float32)
        a2 = (ez / Z).astype(np.float32)

        a0t = a0.reshape(T, P)
        a1t = a1.reshape(T, P)
        a2t = a2.reshape(T, P)
        # combo[c, i, r] = a2[i,r] * (c <= r) + a1[i,r] * (c == r-1)
        combo = a2t[:, None, :] * le[None, :, :]             # [T, c, r]
        combo[:, cc[:-1], cc[1:]] += a1t[:, 1:]
        combo = np.ascontiguousarray(
            combo.transpose(1, 0, 2).reshape(P, T * P)).astype(bf16)

        # auxw[c, i, r]: carries (striu folded: ts[j] feeds all tiles i > j),
        # previous-tile last row for each tile's first row, BOS column term
        auxw = np.zeros((64, T, P), np.float32)
        for i in range(T):
            auxw[:i, i, :] = a2t[i]
            if i >= 1:
                auxw[16 + i - 1, i, 0] = a1t[i, 0]
            auxw[32, i, :] = a0t[i]
        auxw = auxw.reshape(64, T * P).astype(bf16)

        aux0 = np.zeros((P, V), np.float32)
        lastrows = h[b, 127::128, :][:15] * wv               # VA[128j+127]
        aux0[16:16 + 15] = lastrows
        aux0[32] = v_bos

        c0 = np.concatenate([ohc, wvb, aux0], axis=1)        # [P, 3V]

        # x: [p, t, v] so DMA rows are 8KB; xt: [p, rc, kv, r] so each
        # row-chunk slice is 2KB-contiguous
        x2 = np.ascontiguousarray(
            X.reshape(T, P, V).transpose(1, 0, 2).reshape(P, T * V))
        xt4 = np.ascontiguousarray(
            X.T.reshape(2, P, RC, 512).transpose(1, 2, 0, 3))
        wc = np.concatenate([w2t_b, combo], axis=1)          # [P, 8V + T*P]

        in_maps.append({
            "c0": c0.astype(bf16),
            "x": x2.astype(bf16),
            "xt": xt4.astype(bf16),
            "w1t": w1t_b,
            "wc": wc,
            "auxw": auxw,
        })

    mlp_row0 = np.maximum(mlp0 @ w1t, 0.0) @ w2t             # [B, V]
    return in_maps, v_bos, mlp_row0
